# revision 6
# baseline (speedup 1.0000x reference)
"""EdgeConv (GNN message passing) Trainium2 Bass kernel, 8-core SPMD. v2.

Strategy (destination/node sharding, host-side gather, fp16 matmul path):
  * Core r owns destination-node range [r*NPC, (r+1)*NPC) and all edges whose
    col lands there.  Edges sorted by destination, grouped into blocks of 128
    destination nodes, padded to a shared per-block tile count across cores so
    one SPMD program serves all 8 cores.
  * The x[row] gather is done ON HOST (numpy fancy indexing), shipped
    pre-transposed as xrT [128, Ep] fp16 -> the kernel streams it with large
    contiguous HWDGE DMAs.  No indirect DMA, no on-chip transpose.
  * Per tile (128 edges): h = xrT_t.T@W1a + eaT_t.T@W1b  (fp16 matmuls, fp32
    PSUM), LN stats via bn_stats/bn_aggr (DVE), rsqrt via tensor_tensor pow on
    GPSIMD (vpowf; keeps ScalarE inside the gelu table set -> no
    ACT_TABLE_LOAD swaps), exact GELU fused with the LN affine on ScalarE.
  * Aggregation: one-hot S shipped pre-built from host (fp16, streamed like
    the edge data), one matmul per tile: aggH[dst,h] += S.T @ hs accumulated
    in PSUM over the block; per block aggH is copied out and PE-transposed
    back into lhsT layout for the update matmul.  Pad edges get an all-zero
    one-hot column.
  * Tiles processed in PAIRS sharing one [128,1024] PSUM allocation (2 banks)
    so bn_stats handles both tiles in one strided pass.
  * Update MLP per block: u = x@Wua + aggT.T@(W2@Wub) + [cnt|1]@[b2@Wub;bu],
    LN+GELU, +x residual (fp32), store.  Host concatenates 8 slices.
"""
import math
import os
import time
from contextlib import ExitStack

import numpy as np

import concourse.bass as bass
import concourse.bacc as bacc
import concourse.mybir as mybir
import concourse.tile as tile
from concourse.bass_utils import run_bass_kernel_spmd
from concourse.masks import make_identity

# problem constants (hardcoded per harness contract)
N_NODES = 100000
N_EDGES = 1600000
F = 128          # node feature dim (IN_DIM == OUT_DIM)
EDGE_DIM = 32
HID = 256
EPS = 1e-5
N_CORES = 8
P = 128
CH = 32          # tiles per input DMA chunk
LAG = 4          # agg emission lag, in tile pairs

f32 = mybir.dt.float32
f16 = mybir.dt.float16
Alu = mybir.AluOpType
Act = mybir.ActivationFunctionType


def _npc():
    return N_NODES // N_CORES


def _nblocks():
    return math.ceil(_npc() / P)


def _preprocess(x, edge_index, edge_attr):
    """Sort/shard/pad edges by destination; host-gather x rows transposed."""
    NPC, NB = _npc(), _nblocks()
    NODE_PAD = NB * P
    row = np.ascontiguousarray(edge_index[0]).astype(np.int64)
    col = np.ascontiguousarray(edge_index[1]).astype(np.int64)

    order = np.argsort(col, kind="stable")
    col_s = col[order]

    starts = (np.arange(N_CORES)[:, None] * NPC
              + np.arange(NB)[None, :] * P)                      # [R, NB]
    ends = np.minimum(starts + P, (np.arange(N_CORES)[:, None] + 1) * NPC)
    lo = np.searchsorted(col_s, starts.ravel()).reshape(N_CORES, NB)
    hi = np.searchsorted(col_s, ends.ravel()).reshape(N_CORES, NB)
    counts = hi - lo

    bmax = counts.max(axis=0)
    n_tiles = np.maximum(1, -(-bmax // P)).astype(np.int64)
    if n_tiles.sum() % 2:
        n_tiles[-1] += 1                                         # even T_total
    Bj = n_tiles * P
    tile_off = np.concatenate([[0], np.cumsum(n_tiles)])
    T_total = int(tile_off[-1])
    Ep = T_total * P
    dst0 = np.concatenate([[0], np.cumsum(Bj)])

    x16 = x.astype(np.float16)
    ea16 = np.asarray(edge_attr, np.float16)
    node_counts = np.bincount(col, minlength=N_NODES).astype(np.float16)

    per_core = []
    for r in range(N_CORES):
        asm = np.zeros(Ep, np.int64)
        colrel = np.full(Ep, float(P), np.float32)
        for j in range(NB):
            c = counts[r, j]
            dsl = slice(int(dst0[j]), int(dst0[j]) + int(c))
            ssl = order[lo[r, j]:hi[r, j]]
            asm[dsl] = ssl
            colrel[dsl] = (col[ssl] - r * NPC) % P

        xrT = np.ascontiguousarray(x16[row[asm]].T)              # [128, Ep]
        eaT = np.empty((EDGE_DIM + 1, Ep), np.float16)
        eaT[:EDGE_DIM] = ea16[asm].T
        eaT[EDGE_DIM] = 1.0
        # one-hot selection matrices, [128 e, T*128 d] fp16
        colr_t = colrel.reshape(T_total, P)
        S2d = np.ascontiguousarray(
            (colr_t[:, :, None] == np.arange(P, dtype=np.float32))
            .transpose(1, 0, 2).reshape(P, T_total * P)).astype(np.float16)

        xs = np.zeros((NODE_PAD, F), np.float32)
        xs[:NPC] = x[r * NPC:(r + 1) * NPC]
        xnt = np.ascontiguousarray(
            xs.reshape(NB, P, F).transpose(0, 2, 1)).astype(np.float16)
        cnt1 = np.ones((2, NODE_PAD), np.float16)
        cnt1[0] = 0.0
        cnt1[0, :NPC] = node_counts[r * NPC:(r + 1) * NPC]

        per_core.append(dict(xrT=xrT, eaT=eaT, S2d=S2d,
                             xs=xs, xnt=xnt, cnt1=cnt1))

    return per_core, n_tiles.tolist(), tile_off.tolist(), T_total, Ep


def _build_program(n_tiles, tile_off, T_total, Ep):
    NPC, NB = _npc(), _nblocks()
    NODE_PAD = NB * P
    # block id for each tile
    blk_of = np.zeros(T_total, np.int64)
    for j in range(NB):
        blk_of[tile_off[j]:tile_off[j + 1]] = j

    nc = bacc.Bacc("TRN2", target_bir_lowering=False, debug=False,
                   num_devices=N_CORES)

    xr_d = nc.dram_tensor("xrT", [P, Ep], f16, kind="ExternalInput")
    ea_d = nc.dram_tensor("eaT", [EDGE_DIM + 1, Ep], f16, kind="ExternalInput")
    s_d = nc.dram_tensor("S2d", [P, T_total * P], f16, kind="ExternalInput")
    xnt_d = nc.dram_tensor("xnt", [NB, F, P], f16, kind="ExternalInput")
    xs_d = nc.dram_tensor("xs", [NODE_PAD, F], f32, kind="ExternalInput")
    cnt_d = nc.dram_tensor("cnt1", [2, NODE_PAD], f16, kind="ExternalInput")
    w1a_d = nc.dram_tensor("w1a", [F, HID], f16, kind="ExternalInput")
    w1b_d = nc.dram_tensor("w1b", [EDGE_DIM + 1, HID], f16, kind="ExternalInput")
    wua_d = nc.dram_tensor("wua", [F, F], f16, kind="ExternalInput")
    w2u_d = nc.dram_tensor("w2u", [F, HID], f16, kind="ExternalInput")
    bb_d = nc.dram_tensor("b2ubu", [2, F], f16, kind="ExternalInput")
    out_d = nc.dram_tensor("out", [NODE_PAD, F], f32, kind="ExternalOutput")

    n_pairs = T_total // 2

    with tile.TileContext(nc) as tc, ExitStack() as ctx:
        cb = ctx.enter_context(tc.tile_pool(name="cb", bufs=1))
        xrp = ctx.enter_context(tc.tile_pool(name="xrp", bufs=2))
        eap = ctx.enter_context(tc.tile_pool(name="eap", bufs=2))
        ssp = ctx.enter_context(tc.tile_pool(name="ssp", bufs=2))
        hsp = ctx.enter_context(tc.tile_pool(name="hsp", bufs=10))
        stp = ctx.enter_context(tc.tile_pool(name="stp", bufs=4))
        blk = ctx.enter_context(tc.tile_pool(name="blk", bufs=2))
        ublk = ctx.enter_context(tc.tile_pool(name="ublk", bufs=2))
        ps_h = ctx.enter_context(tc.tile_pool(name="ps_h", bufs=4, space="PSUM"))
        ps_agg = ctx.enter_context(tc.tile_pool(name="ps_agg", bufs=2, space="PSUM"))
        ps_t = ctx.enter_context(tc.tile_pool(name="ps_t", bufs=1, space="PSUM"))
        ps_u = ctx.enter_context(tc.tile_pool(name="ps_u", bufs=1, space="PSUM"))

        # ---- constants / whole-run loads ----
        ident = cb.tile([P, P], f16)
        make_identity(nc, ident[:])
        epsb2 = cb.tile([P, 2], f32)
        nc.vector.memset(epsb2[:], EPS)
        mh2 = cb.tile([P, 2], f32)
        nc.vector.memset(mh2[:], -0.5)

        w1a_s = cb.tile([F, HID], f16)
        nc.sync.dma_start(w1a_s[:], w1a_d.ap())
        w1b_s = cb.tile([EDGE_DIM + 1, HID], f16)
        nc.sync.dma_start(w1b_s[:], w1b_d.ap())
        wua_s = cb.tile([F, F], f16)
        nc.sync.dma_start(wua_s[:], wua_d.ap())
        w2u_s = cb.tile([F, HID], f16)
        nc.sync.dma_start(w2u_s[:], w2u_d.ap())
        bb_s = cb.tile([2, F], f16)
        nc.sync.dma_start(bb_s[:], bb_d.ap())
        cnt_s = cb.tile([2, NODE_PAD], f16)
        nc.sync.dma_start(cnt_s[:], cnt_d.ap())

        # rolling state
        state = dict(xr=None, ea=None)
        hp_tiles = [None] * n_pairs      # hpair psum tiles by pair index
        hs_tiles = [None] * T_total
        S_chunks = [None] * T_total      # (chunk tile, col offset) per tile
        blk_in = [None] * NB             # (xnt_s, xn_s) per block

        def _ap3(base, off, d1, n1, d2, n2):
            """[128, n1, n2] strided view of a tile AP."""
            return bass.AP(base.tensor, base.offset + off,
                           [list(base.ap[0]), [d1, n1], [d2, n2]])

        def front(m):
            t0, t1 = 2 * m, 2 * m + 1
            for t in (t0, t1):
                c, ci = divmod(t, CH)
                if ci == 0:
                    w = min(CH, T_total - c * CH) * P
                    xr = xrp.tile([P, CH * P], f16, tag="xr")
                    nc.sync.dma_start(xr[:, :w], xr_d.ap()[:, c * CH * P:c * CH * P + w])
                    ea = eap.tile([EDGE_DIM + 1, CH * P], f16, tag="ea")
                    nc.sync.dma_start(ea[:, :w], ea_d.ap()[:, c * CH * P:c * CH * P + w])
                    sc = ssp.tile([P, CH * P], f16, tag="sc")
                    nc.sync.dma_start(sc[:, :w], s_d.ap()[:, c * CH * P:c * CH * P + w])
                    state["xr"], state["ea"], state["sc"] = xr, ea, sc
                # prefetch per-block update inputs when a block begins
                j = int(blk_of[t])
                if t == tile_off[j] and blk_in[j] is None:
                    xnt_s = blk.tile([F, P], f16, tag="xnt")
                    nc.sync.dma_start(xnt_s[:], xnt_d.ap()[j])
                    xn_s = blk.tile([P, F], f32, tag="xn")
                    nc.sync.dma_start(xn_s[:], xs_d.ap()[j * P:(j + 1) * P, :])
                    blk_in[j] = (xnt_s, xn_s)
                S_chunks[t] = (state["sc"], (t % CH) * P)

            hp = ps_h.tile([P, 512], f32, space="PSUM", tag="hp")
            hp_tiles[m] = hp
            for k, t in enumerate((t0, t1)):
                ci = t % CH
                off = HID * k
                nc.tensor.matmul(out=hp[:, off:off + HID],
                                 lhsT=state["xr"][:, ci * P:(ci + 1) * P],
                                 rhs=w1a_s[:], start=(k == 0), stop=False)
                nc.tensor.matmul(out=hp[:, off:off + HID],
                                 lhsT=state["ea"][:, ci * P:(ci + 1) * P],
                                 rhs=w1b_s[:], start=False, stop=(k == 1))

            # LN stats, one bn_stats per tile (HW BNStats writes 6/partition)
            st2 = stp.tile([P, 14], f32, tag="st2")
            nc.vector.bn_stats(st2[:, 0:6], hp[:, 0:HID])
            nc.vector.bn_stats(st2[:, 7:13], hp[:, HID:2 * HID])
            mv2 = stp.tile([P, 4], f32, tag="mv2")
            nc.vector.bn_aggr(mv2[:, 0:2], st2[:, 0:6])
            nc.vector.bn_aggr(mv2[:, 2:4], st2[:, 7:13])
            # r = (var+eps)^-0.5 on gpsimd (vpowf); var at mv2[:, {1,3}]
            vpe = stp.tile([P, 2], f32, tag="vpe")
            mvb = mv2[:]
            var2 = bass.AP(mvb.tensor, mvb.offset + 1, [list(mvb.ap[0]), [2, 2]])
            mean2 = bass.AP(mvb.tensor, mvb.offset + 0, [list(mvb.ap[0]), [2, 2]])
            nc.gpsimd.tensor_tensor(out=vpe[:], in0=var2, in1=epsb2[:], op=Alu.add)
            r2 = stp.tile([P, 2], f32, tag="r2")
            nc.gpsimd.tensor_tensor(out=r2[:], in0=vpe[:], in1=mh2[:], op=Alu.pow)
            nmr2 = stp.tile([P, 2], f32, tag="nmr2")
            nc.vector.scalar_tensor_tensor(out=nmr2[:], in0=mean2, scalar=-1.0,
                                           in1=r2[:], op0=Alu.mult, op1=Alu.mult)

            for k, t in enumerate((t0, t1)):
                off = HID * k
                hs = hsp.tile([P, HID], f16, tag="hs")
                nc.scalar.activation(hs[:], hp[:, off:off + HID], Act.Gelu,
                                     bias=nmr2[:, k:k + 1], scale=r2[:, k:k + 1])
                hs_tiles[t] = hs

        agg_state = dict(aggh=None)

        def back(m):
            for t in (2 * m, 2 * m + 1):
                j = int(blk_of[t])
                ti = t - tile_off[j]
                nt = n_tiles[j]
                if ti == 0:
                    agg_state["aggh"] = ps_agg.tile([P, HID], f32, space="PSUM", tag="aggh", name="aggh")
                hs = hs_tiles[t]
                sc, so = S_chunks[t]
                nc.tensor.matmul(out=agg_state["aggh"][:], lhsT=sc[:, so:so + P],
                                 rhs=hs[:], start=(ti == 0), stop=(ti == nt - 1))
                hs_tiles[t] = S_chunks[t] = None
                if ti == nt - 1:
                    block_end(j)

        def block_end(j):
            aggsb = ublk.tile([P, HID], f16, tag="aggsb")
            nc.scalar.copy(aggsb[:, 0:P], agg_state["aggh"][:, 0:P])
            nc.vector.tensor_copy(out=aggsb[:, P:HID], in_=agg_state["aggh"][:, P:HID])
            # transpose [dst, h] -> [h, dst] halves for the update lhsT
            tp = ps_t.tile([P, P], f16, space="PSUM", tag="tp")
            nc.tensor.transpose(out=tp[:], in_=aggsb[:, 0:P], identity=ident[:])
            aggt0 = ublk.tile([P, P], f16, tag="aggt0")
            nc.vector.tensor_copy(out=aggt0[:], in_=tp[:])
            tp2 = ps_t.tile([P, P], f16, space="PSUM", tag="tp")
            nc.tensor.transpose(out=tp2[:], in_=aggsb[:, P:HID], identity=ident[:])
            aggt1 = ublk.tile([P, P], f16, tag="aggt1")
            nc.scalar.copy(aggt1[:], tp2[:])

            xnt_s, xn_s = blk_in[j]
            u_ps = ps_u.tile([P, F], f32, space="PSUM", tag="u")
            nc.tensor.matmul(out=u_ps[:], lhsT=xnt_s[:], rhs=wua_s[:],
                             start=True, stop=False)
            nc.tensor.matmul(out=u_ps[:], lhsT=aggt0[:], rhs=w2u_s[:, 0:F],
                             start=False, stop=False)
            nc.tensor.matmul(out=u_ps[:], lhsT=aggt1[:], rhs=w2u_s[:, F:HID],
                             start=False, stop=False)
            nc.tensor.matmul(out=u_ps[:], lhsT=cnt_s[:, j * P:(j + 1) * P],
                             rhs=bb_s[:], start=False, stop=True)

            stu = stp.tile([P, 6], f32, tag="stu")
            nc.vector.bn_stats(stu[:], u_ps[:])
            mvu = stp.tile([P, 2], f32, tag="mvu")
            nc.vector.bn_aggr(mvu[:], stu[:])
            vpu = stp.tile([P, 1], f32, tag="vpu")
            nc.gpsimd.tensor_tensor(out=vpu[:], in0=mvu[:, 1:2],
                                    in1=epsb2[:, 0:1], op=Alu.add)
            ru = stp.tile([P, 1], f32, tag="ru")
            nc.gpsimd.tensor_tensor(out=ru[:], in0=vpu[:], in1=mh2[:, 0:1],
                                    op=Alu.pow)
            nmru = stp.tile([P, 1], f32, tag="nmru")
            nc.vector.scalar_tensor_tensor(out=nmru[:], in0=mvu[:, 0:1],
                                           scalar=-1.0, in1=ru[:],
                                           op0=Alu.mult, op1=Alu.mult)
            us = ublk.tile([P, F], f32, tag="us")
            nc.scalar.activation(us[:], u_ps[:], Act.Gelu,
                                 bias=nmru[:, 0:1], scale=ru[:, 0:1])
            uo = ublk.tile([P, F], f32, tag="uo")
            nc.vector.tensor_tensor(out=uo[:], in0=us[:], in1=xn_s[:], op=Alu.add)
            nc.sync.dma_start(out_d.ap()[j * P:(j + 1) * P, :], uo[:])
            blk_in[j] = None

        for m in range(n_pairs):
            front(m)
            if m >= LAG:
                back(m - LAG)
                hp_tiles[m - LAG] = None
        for m in range(max(0, n_pairs - LAG), n_pairs):
            back(m)
            hp_tiles[m] = None

    nc.compile()
    return nc


def _fold_weights(W1, b1, W2, b2, Wu, bu):
    w1 = np.concatenate([W1, b1[None, :]], 0).astype(np.float32)   # [161, 256]
    wub = Wu[F:]                                                   # [128, 128]
    W2u = (W2 @ wub).astype(np.float32)                            # [256, 128]
    return dict(
        w1a=np.ascontiguousarray(w1[:F]).astype(np.float16),
        w1b=np.ascontiguousarray(w1[F:]).astype(np.float16),
        wua=np.ascontiguousarray(Wu[:F]).astype(np.float16),
        w2u=np.ascontiguousarray(
            np.concatenate([W2u[:F], W2u[F:]], 1)).astype(np.float16),
        b2ubu=np.stack([b2 @ wub, bu]).astype(np.float16),
    )


def run(inputs, trace=False, tmpdir=None):
    x = np.asarray(inputs["x"], np.float32)
    g1, be1 = np.asarray(inputs["g1"]), np.asarray(inputs["be1"])
    gu, beu = np.asarray(inputs["gu"]), np.asarray(inputs["beu"])
    if not (np.all(g1 == 1) and np.all(be1 == 0) and np.all(gu == 1)
            and np.all(beu == 0)):
        raise NotImplementedError("nontrivial LayerNorm affine not supported")

    t0 = time.time()
    per_core, n_tiles, tile_off, T_total, Ep = _preprocess(
        x, inputs["edge_index"], inputs["edge_attr"])
    shared = _fold_weights(
        np.asarray(inputs["W1"], np.float32), np.asarray(inputs["b1"], np.float32),
        np.asarray(inputs["W2"], np.float32), np.asarray(inputs["b2"], np.float32),
        np.asarray(inputs["Wu"], np.float32), np.asarray(inputs["bu"], np.float32))
    in_maps = [{**shared, **pc} for pc in per_core]
    t1 = time.time()

    nc = _build_program(n_tiles, tile_off, T_total, Ep)
    t2 = time.time()

    res = run_bass_kernel_spmd(nc, in_maps, core_ids=list(range(N_CORES)),
                               trace=trace, tmpdir=tmpdir,
                               trace_cores=[0] if trace else None)
    t3 = time.time()
    if os.environ.get("KERNEL_VERBOSE"):
        print(f"preprocess {t1-t0:.1f}s  build+compile {t2-t1:.1f}s  run {t3-t2:.1f}s")

    NPC = _npc()
    out = np.concatenate([res.results[r]["out"][:NPC] for r in range(N_CORES)], 0)
    return out, res


def kernel(**inputs):
    out, _ = run(inputs, trace=False)
    return out


# revision 7
# speedup vs baseline: 1.0735x; 1.0735x over previous
"""EdgeConv (GNN message passing) Trainium2 Bass kernel, 8-core SPMD. v2.

Strategy (destination/node sharding, host-side gather, fp16 matmul path):
  * Core r owns destination-node range [r*NPC, (r+1)*NPC) and all edges whose
    col lands there.  Edges sorted by destination, grouped into blocks of 128
    destination nodes, padded to a shared per-block tile count across cores so
    one SPMD program serves all 8 cores.
  * The x[row] gather is done ON HOST (numpy fancy indexing), shipped
    pre-transposed as xrT [128, Ep] fp16 -> the kernel streams it with large
    contiguous HWDGE DMAs.  No indirect DMA, no on-chip transpose.
  * Per tile (128 edges): h = xrT_t.T@W1a + eaT_t.T@W1b  (fp16 matmuls, fp32
    PSUM), LN stats via bn_stats/bn_aggr (DVE), rsqrt via tensor_tensor pow on
    GPSIMD (vpowf; keeps ScalarE inside the gelu table set -> no
    ACT_TABLE_LOAD swaps), exact GELU fused with the LN affine on ScalarE.
  * Aggregation: one-hot S shipped pre-built from host (fp16, streamed like
    the edge data), aggT[h,dst] += hs[:,half].T @ S accumulated in PSUM over
    the block (two matmuls, alternating banks).  Pad edges get an all-zero
    one-hot column.  ALL matmuls keep contraction K=128 (edge_attr, counts
    and their weight blocks are zero-padded to 128 rows) -- alternating K
    reconfigures the PE array and costs ~280ns per matmul.
  * Tiles processed in PAIRS sharing one [128,1024] PSUM allocation (2 banks)
    so bn_stats handles both tiles in one strided pass.
  * Update MLP per block: u = x@Wua + aggT.T@(W2@Wub) + [cnt|1]@[b2@Wub;bu],
    LN+GELU, +x residual (fp32), store.  Host concatenates 8 slices.
"""
import math
import os
import time
from contextlib import ExitStack

import numpy as np

import concourse.bass as bass
import concourse.bacc as bacc
import concourse.mybir as mybir
import concourse.tile as tile
from concourse.bass_utils import run_bass_kernel_spmd
from concourse.masks import make_identity

# problem constants (hardcoded per harness contract)
N_NODES = 100000
N_EDGES = 1600000
F = 128          # node feature dim (IN_DIM == OUT_DIM)
EDGE_DIM = 32
HID = 256
EPS = 1e-5
N_CORES = 8
P = 128
CH = 32          # tiles per input DMA chunk
LAG = 4          # agg emission lag, in tile pairs

f32 = mybir.dt.float32
f16 = mybir.dt.float16
Alu = mybir.AluOpType
Act = mybir.ActivationFunctionType


def _npc():
    return N_NODES // N_CORES


def _nblocks():
    return math.ceil(_npc() / P)


def _preprocess(x, edge_index, edge_attr):
    """Sort/shard/pad edges by destination; host-gather x rows transposed."""
    NPC, NB = _npc(), _nblocks()
    NODE_PAD = NB * P
    row = np.ascontiguousarray(edge_index[0]).astype(np.int64)
    col = np.ascontiguousarray(edge_index[1]).astype(np.int64)

    order = np.argsort(col, kind="stable")
    col_s = col[order]

    starts = (np.arange(N_CORES)[:, None] * NPC
              + np.arange(NB)[None, :] * P)                      # [R, NB]
    ends = np.minimum(starts + P, (np.arange(N_CORES)[:, None] + 1) * NPC)
    lo = np.searchsorted(col_s, starts.ravel()).reshape(N_CORES, NB)
    hi = np.searchsorted(col_s, ends.ravel()).reshape(N_CORES, NB)
    counts = hi - lo

    bmax = counts.max(axis=0)
    n_tiles = np.maximum(1, -(-bmax // P)).astype(np.int64)
    if n_tiles.sum() % 2:
        n_tiles[-1] += 1                                         # even T_total
    Bj = n_tiles * P
    tile_off = np.concatenate([[0], np.cumsum(n_tiles)])
    T_total = int(tile_off[-1])
    Ep = T_total * P
    dst0 = np.concatenate([[0], np.cumsum(Bj)])

    x16 = x.astype(np.float16)
    ea16 = np.asarray(edge_attr, np.float16)
    node_counts = np.bincount(col, minlength=N_NODES).astype(np.float16)

    per_core = []
    for r in range(N_CORES):
        asm = np.zeros(Ep, np.int64)
        colrel = np.full(Ep, float(P), np.float32)
        for j in range(NB):
            c = counts[r, j]
            dsl = slice(int(dst0[j]), int(dst0[j]) + int(c))
            ssl = order[lo[r, j]:hi[r, j]]
            asm[dsl] = ssl
            colrel[dsl] = (col[ssl] - r * NPC) % P

        xrT = np.ascontiguousarray(x16[row[asm]].T)              # [128, Ep]
        eaT = np.zeros((P, Ep), np.float16)
        eaT[:EDGE_DIM] = ea16[asm].T
        eaT[EDGE_DIM] = 1.0
        # one-hot selection matrices, [128 e, T*128 d] fp16
        colr_t = colrel.reshape(T_total, P)
        S2d = np.ascontiguousarray(
            (colr_t[:, :, None] == np.arange(P, dtype=np.float32))
            .transpose(1, 0, 2).reshape(P, T_total * P)).astype(np.float16)

        xs = np.zeros((NODE_PAD, F), np.float32)
        xs[:NPC] = x[r * NPC:(r + 1) * NPC]
        xnt = np.ascontiguousarray(
            xs.reshape(NB, P, F).transpose(0, 2, 1)).astype(np.float16)
        cnt1 = np.zeros((P, NODE_PAD), np.float16)
        cnt1[0, :NPC] = node_counts[r * NPC:(r + 1) * NPC]
        cnt1[1] = 1.0

        per_core.append(dict(xrT=xrT, eaT=eaT, S2d=S2d,
                             xs=xs, xnt=xnt, cnt1=cnt1))

    return per_core, n_tiles.tolist(), tile_off.tolist(), T_total, Ep


def _build_program(n_tiles, tile_off, T_total, Ep):
    NPC, NB = _npc(), _nblocks()
    NODE_PAD = NB * P
    # block id for each tile
    blk_of = np.zeros(T_total, np.int64)
    for j in range(NB):
        blk_of[tile_off[j]:tile_off[j + 1]] = j

    nc = bacc.Bacc("TRN2", target_bir_lowering=False, debug=False,
                   num_devices=N_CORES)

    xr_d = nc.dram_tensor("xrT", [P, Ep], f16, kind="ExternalInput")
    ea_d = nc.dram_tensor("eaT", [P, Ep], f16, kind="ExternalInput")
    s_d = nc.dram_tensor("S2d", [P, T_total * P], f16, kind="ExternalInput")
    xnt_d = nc.dram_tensor("xnt", [NB, F, P], f16, kind="ExternalInput")
    xs_d = nc.dram_tensor("xs", [NODE_PAD, F], f32, kind="ExternalInput")
    cnt_d = nc.dram_tensor("cnt1", [P, NODE_PAD], f16, kind="ExternalInput")
    w1a_d = nc.dram_tensor("w1a", [F, HID], f16, kind="ExternalInput")
    w1b_d = nc.dram_tensor("w1b", [P, HID], f16, kind="ExternalInput")
    wua_d = nc.dram_tensor("wua", [F, F], f16, kind="ExternalInput")
    w2u_d = nc.dram_tensor("w2u", [F, HID], f16, kind="ExternalInput")
    bb_d = nc.dram_tensor("b2ubu", [P, F], f16, kind="ExternalInput")
    out_d = nc.dram_tensor("out", [NODE_PAD, F], f32, kind="ExternalOutput")

    n_pairs = T_total // 2

    with tile.TileContext(nc) as tc, ExitStack() as ctx:
        cb = ctx.enter_context(tc.tile_pool(name="cb", bufs=1))
        xrp = ctx.enter_context(tc.tile_pool(name="xrp", bufs=2))
        eap = ctx.enter_context(tc.tile_pool(name="eap", bufs=2))
        ssp = ctx.enter_context(tc.tile_pool(name="ssp", bufs=2))
        hsp = ctx.enter_context(tc.tile_pool(name="hsp", bufs=10))
        stp = ctx.enter_context(tc.tile_pool(name="stp", bufs=4))
        blk = ctx.enter_context(tc.tile_pool(name="blk", bufs=2))
        ublk = ctx.enter_context(tc.tile_pool(name="ublk", bufs=2))
        ps_h = ctx.enter_context(tc.tile_pool(name="ps_h", bufs=4, space="PSUM"))
        ps_agg = ctx.enter_context(tc.tile_pool(name="ps_agg", bufs=1, space="PSUM"))
        ps_u = ctx.enter_context(tc.tile_pool(name="ps_u", bufs=1, space="PSUM"))

        # ---- constants / whole-run loads ----
        epsb2 = cb.tile([P, 2], f32)
        nc.vector.memset(epsb2[:], EPS)
        mh2 = cb.tile([P, 2], f32)
        nc.vector.memset(mh2[:], -0.5)

        w1a_s = cb.tile([F, HID], f16)
        nc.sync.dma_start(w1a_s[:], w1a_d.ap())
        w1b_s = cb.tile([P, HID], f16)
        nc.sync.dma_start(w1b_s[:], w1b_d.ap())
        wua_s = cb.tile([F, F], f16)
        nc.sync.dma_start(wua_s[:], wua_d.ap())
        w2u_s = cb.tile([F, HID], f16)
        nc.sync.dma_start(w2u_s[:], w2u_d.ap())
        bb_s = cb.tile([P, F], f16)
        nc.sync.dma_start(bb_s[:], bb_d.ap())
        cnt_s = cb.tile([P, NODE_PAD], f16)
        nc.sync.dma_start(cnt_s[:], cnt_d.ap())

        # rolling state
        state = dict(xr=None, ea=None)
        hp_tiles = [None] * n_pairs      # hpair psum tiles by pair index
        hs_tiles = [None] * T_total
        S_chunks = [None] * T_total      # (chunk tile, col offset) per tile
        blk_in = [None] * NB             # (xnt_s, xn_s) per block

        def _ap3(base, off, d1, n1, d2, n2):
            """[128, n1, n2] strided view of a tile AP."""
            return bass.AP(base.tensor, base.offset + off,
                           [list(base.ap[0]), [d1, n1], [d2, n2]])

        def front(m):
            t0, t1 = 2 * m, 2 * m + 1
            for t in (t0, t1):
                c, ci = divmod(t, CH)
                if ci == 0:
                    w = min(CH, T_total - c * CH) * P
                    xr = xrp.tile([P, CH * P], f16, tag="xr")
                    nc.sync.dma_start(xr[:, :w], xr_d.ap()[:, c * CH * P:c * CH * P + w])
                    ea = eap.tile([P, CH * P], f16, tag="ea")
                    nc.sync.dma_start(ea[:, :w], ea_d.ap()[:, c * CH * P:c * CH * P + w])
                    sc = ssp.tile([P, CH * P], f16, tag="sc")
                    nc.sync.dma_start(sc[:, :w], s_d.ap()[:, c * CH * P:c * CH * P + w])
                    state["xr"], state["ea"], state["sc"] = xr, ea, sc
                # prefetch per-block update inputs when a block begins
                j = int(blk_of[t])
                if t == tile_off[j] and blk_in[j] is None:
                    xnt_s = blk.tile([F, P], f16, tag="xnt")
                    nc.sync.dma_start(xnt_s[:], xnt_d.ap()[j])
                    xn_s = blk.tile([P, F], f32, tag="xn")
                    nc.sync.dma_start(xn_s[:], xs_d.ap()[j * P:(j + 1) * P, :])
                    blk_in[j] = (xnt_s, xn_s)
                S_chunks[t] = (state["sc"], (t % CH) * P)

            hp = ps_h.tile([P, 512], f32, space="PSUM", tag="hp")
            hp_tiles[m] = hp
            for k, t in enumerate((t0, t1)):
                ci = t % CH
                off = HID * k
                nc.tensor.matmul(out=hp[:, off:off + HID],
                                 lhsT=state["xr"][:, ci * P:(ci + 1) * P],
                                 rhs=w1a_s[:], start=(k == 0), stop=False)
                nc.tensor.matmul(out=hp[:, off:off + HID],
                                 lhsT=state["ea"][:, ci * P:(ci + 1) * P],
                                 rhs=w1b_s[:], start=False, stop=(k == 1))

            # LN stats, one bn_stats per tile (HW BNStats writes 6/partition)
            st2 = stp.tile([P, 14], f32, tag="st2")
            nc.vector.bn_stats(st2[:, 0:6], hp[:, 0:HID])
            nc.vector.bn_stats(st2[:, 7:13], hp[:, HID:2 * HID])
            mv2 = stp.tile([P, 4], f32, tag="mv2")
            nc.vector.bn_aggr(mv2[:, 0:2], st2[:, 0:6])
            nc.vector.bn_aggr(mv2[:, 2:4], st2[:, 7:13])
            # r = (var+eps)^-0.5 on gpsimd (vpowf); var at mv2[:, {1,3}]
            vpe = stp.tile([P, 2], f32, tag="vpe")
            mvb = mv2[:]
            var2 = bass.AP(mvb.tensor, mvb.offset + 1, [list(mvb.ap[0]), [2, 2]])
            mean2 = bass.AP(mvb.tensor, mvb.offset + 0, [list(mvb.ap[0]), [2, 2]])
            nc.gpsimd.tensor_tensor(out=vpe[:], in0=var2, in1=epsb2[:], op=Alu.add)
            r2 = stp.tile([P, 2], f32, tag="r2")
            nc.gpsimd.tensor_tensor(out=r2[:], in0=vpe[:], in1=mh2[:], op=Alu.pow)
            nmr2 = stp.tile([P, 2], f32, tag="nmr2")
            nc.vector.scalar_tensor_tensor(out=nmr2[:], in0=mean2, scalar=-1.0,
                                           in1=r2[:], op0=Alu.mult, op1=Alu.mult)

            for k, t in enumerate((t0, t1)):
                off = HID * k
                hs = hsp.tile([P, HID], f16, tag="hs")
                nc.scalar.activation(hs[:], hp[:, off:off + HID], Act.Gelu,
                                     bias=nmr2[:, k:k + 1], scale=r2[:, k:k + 1])
                hs_tiles[t] = hs

        agg_state = dict(agg0=None, agg1=None)

        def back(m):
            for t in (2 * m, 2 * m + 1):
                j = int(blk_of[t])
                ti = t - tile_off[j]
                nt = n_tiles[j]
                if ti == 0:
                    agg_state["agg0"] = ps_agg.tile([P, P], f32, space="PSUM", tag="agg0", name="agg0")
                    agg_state["agg1"] = ps_agg.tile([P, P], f32, space="PSUM", tag="agg1", name="agg1")
                hs = hs_tiles[t]
                sc, so = S_chunks[t]
                nc.tensor.matmul(out=agg_state["agg0"][:], lhsT=hs[:, 0:P],
                                 rhs=sc[:, so:so + P], start=(ti == 0),
                                 stop=(ti == nt - 1))
                nc.tensor.matmul(out=agg_state["agg1"][:], lhsT=hs[:, P:HID],
                                 rhs=sc[:, so:so + P], start=(ti == 0),
                                 stop=(ti == nt - 1))
                hs_tiles[t] = S_chunks[t] = None
                if ti == nt - 1:
                    block_end(j)

        def block_end(j):
            aggsb = ublk.tile([P, HID], f16, tag="aggsb")
            nc.scalar.copy(aggsb[:, 0:P], agg_state["agg0"][:])
            nc.vector.tensor_copy(out=aggsb[:, P:HID], in_=agg_state["agg1"][:])

            xnt_s, xn_s = blk_in[j]
            u_ps = ps_u.tile([P, F], f32, space="PSUM", tag="u")
            nc.tensor.matmul(out=u_ps[:], lhsT=xnt_s[:], rhs=wua_s[:],
                             start=True, stop=False)
            nc.tensor.matmul(out=u_ps[:], lhsT=aggsb[:, 0:P], rhs=w2u_s[:, 0:F],
                             start=False, stop=False)
            nc.tensor.matmul(out=u_ps[:], lhsT=aggsb[:, P:HID], rhs=w2u_s[:, F:HID],
                             start=False, stop=False)
            nc.tensor.matmul(out=u_ps[:], lhsT=cnt_s[:, j * P:(j + 1) * P],
                             rhs=bb_s[:], start=False, stop=True)

            stu = stp.tile([P, 6], f32, tag="stu")
            nc.vector.bn_stats(stu[:], u_ps[:])
            mvu = stp.tile([P, 2], f32, tag="mvu")
            nc.vector.bn_aggr(mvu[:], stu[:])
            vpu = stp.tile([P, 1], f32, tag="vpu")
            nc.gpsimd.tensor_tensor(out=vpu[:], in0=mvu[:, 1:2],
                                    in1=epsb2[:, 0:1], op=Alu.add)
            ru = stp.tile([P, 1], f32, tag="ru")
            nc.gpsimd.tensor_tensor(out=ru[:], in0=vpu[:], in1=mh2[:, 0:1],
                                    op=Alu.pow)
            nmru = stp.tile([P, 1], f32, tag="nmru")
            nc.vector.scalar_tensor_tensor(out=nmru[:], in0=mvu[:, 0:1],
                                           scalar=-1.0, in1=ru[:],
                                           op0=Alu.mult, op1=Alu.mult)
            us = ublk.tile([P, F], f32, tag="us")
            nc.scalar.activation(us[:], u_ps[:], Act.Gelu,
                                 bias=nmru[:, 0:1], scale=ru[:, 0:1])
            uo = ublk.tile([P, F], f32, tag="uo")
            nc.vector.tensor_tensor(out=uo[:], in0=us[:], in1=xn_s[:], op=Alu.add)
            nc.sync.dma_start(out_d.ap()[j * P:(j + 1) * P, :], uo[:])
            blk_in[j] = None

        for m in range(n_pairs):
            front(m)
            if m >= LAG:
                back(m - LAG)
                hp_tiles[m - LAG] = None
        for m in range(max(0, n_pairs - LAG), n_pairs):
            back(m)
            hp_tiles[m] = None

    nc.compile()
    return nc


def _fold_weights(W1, b1, W2, b2, Wu, bu):
    w1 = np.concatenate([W1, b1[None, :]], 0).astype(np.float32)   # [161, 256]
    wub = Wu[F:]                                                   # [128, 128]
    W2u = (W2 @ wub).astype(np.float32)                            # [256, 128]
    w1b = np.zeros((P, HID), np.float32)
    w1b[:EDGE_DIM + 1] = w1[F:]
    bb = np.zeros((P, F), np.float32)
    bb[0] = b2 @ wub
    bb[1] = bu
    return dict(
        w1a=np.ascontiguousarray(w1[:F]).astype(np.float16),
        w1b=w1b.astype(np.float16),
        wua=np.ascontiguousarray(Wu[:F]).astype(np.float16),
        w2u=np.ascontiguousarray(
            np.concatenate([W2u[:F], W2u[F:]], 1)).astype(np.float16),
        b2ubu=bb.astype(np.float16),
    )


def run(inputs, trace=False, tmpdir=None):
    x = np.asarray(inputs["x"], np.float32)
    g1, be1 = np.asarray(inputs["g1"]), np.asarray(inputs["be1"])
    gu, beu = np.asarray(inputs["gu"]), np.asarray(inputs["beu"])
    if not (np.all(g1 == 1) and np.all(be1 == 0) and np.all(gu == 1)
            and np.all(beu == 0)):
        raise NotImplementedError("nontrivial LayerNorm affine not supported")

    t0 = time.time()
    per_core, n_tiles, tile_off, T_total, Ep = _preprocess(
        x, inputs["edge_index"], inputs["edge_attr"])
    shared = _fold_weights(
        np.asarray(inputs["W1"], np.float32), np.asarray(inputs["b1"], np.float32),
        np.asarray(inputs["W2"], np.float32), np.asarray(inputs["b2"], np.float32),
        np.asarray(inputs["Wu"], np.float32), np.asarray(inputs["bu"], np.float32))
    in_maps = [{**shared, **pc} for pc in per_core]
    t1 = time.time()

    nc = _build_program(n_tiles, tile_off, T_total, Ep)
    t2 = time.time()

    res = run_bass_kernel_spmd(nc, in_maps, core_ids=list(range(N_CORES)),
                               trace=trace, tmpdir=tmpdir,
                               trace_cores=[0] if trace else None)
    t3 = time.time()
    if os.environ.get("KERNEL_VERBOSE"):
        print(f"preprocess {t1-t0:.1f}s  build+compile {t2-t1:.1f}s  run {t3-t2:.1f}s")

    NPC = _npc()
    out = np.concatenate([res.results[r]["out"][:NPC] for r in range(N_CORES)], 0)
    return out, res


def kernel(**inputs):
    out, _ = run(inputs, trace=False)
    return out
